# revision 1
# baseline (speedup 1.0000x reference)
"""DeepseekV3 top-k router kernel for Trainium2 (Bass/Tile), 8-core SPMD.

Reference computation (per token, 256 experts):
  s    = sigmoid(logits)            computed as 1/(1+exp(-x)) with the DVE's
                                    bit-exact reciprocal -- bitwise identical
                                    to jax-on-neuron's logistic lowering
  s4c  = s + correction_bias
  group scores = sum of top-2 of s4c within each of 8 groups of 32
  top-4 groups -> mask -> masked s4c
  top-8 of masked s4c -> (indices, values)   [DVE Max/MaxIndex: jax tie rules]
  weights = s at those indices, normalized to sum 2.5

Sharding: data-parallel on the token dim across 8 cores (16384 tokens each);
the 256-entry correction bias is replicated. Layout: one token per SBUF
partition, its 256 expert scores along the free dim; 128 tokens per tile,
8 tiles per "supertile" so the narrow [P,8]-ish stages run batched.

Engine split: ACT does exp and the +1; the Pool (gpsimd) engine does the
bias add; DVE does the reciprocal, group maxes (the second max via a fused
custom DVE op), masking, and the Max8/FindIndex8/MatchReplace extraction.
"""
import numpy as np

import concourse.bass as bass
import concourse.bacc as bacc
import concourse.mybir as mybir
from concourse.tile import TileContext
from concourse.bass_utils import run_bass_kernel_spmd

F32 = mybir.dt.float32
U32 = mybir.dt.uint32

T_FULL = 131072
E = 256
N_CORES = 8
T_CORE = T_FULL // N_CORES      # 16384
P = 128                         # tokens per tile (one per partition)
N_TILES = T_CORE // P           # 128
B = 8                           # tiles per supertile
N_SUPER = N_TILES // B
G = 8                           # expert groups
EG = E // G                     # experts per group
BIG = 1.0e30

LAST_EXEC_NS = None
LAST_RESULTS = None

_EQNEG = None


def _get_eqneg():
    """Fused custom DVE op: out = (in0 == in1) ? -FLT_MAX : in0 (one pass,
    replaces a tensor_tensor(is_equal) + scalar_tensor_tensor pair)."""
    global _EQNEG
    if _EQNEG is None:
        from concourse.dve_ops import (DveOp, OPS, get_dve_sub_opcode,
                                       has_src1)
        from concourse.dve_spec import Spec, Src0, Src1, MaxNeg, select, eq, lower
        from concourse.dve_uop import DveOpSpec
        import concourse.dve_ops as dve_ops_mod

        spec = Spec(
            body=select(eq(Src0, Src1), MaxNeg, Src0),
            reference=lambda in0, in1, s0, s1, imm2: np.where(
                in0 == in1, np.float32(-3.4028234663852886e38), in0
            ).astype(np.float32),
        )
        op = DveOp("RTR_EQNEG", spec, subdim=False, uops_sha={})
        OPS.append(op)
        dve_ops_mod.CUSTOM_DVE_SPECS[op.name] = op.spec
        dve_ops_mod._SUB_OPCODE_FOR_NAME[op.name] = (
            dve_ops_mod._CUSTOM_DVE_ROW_BASE + len(OPS) - 1)
        assert dve_ops_mod._SUB_OPCODE_FOR_NAME[op.name] < 0x20
        for ver in ("v3", "v4"):
            tmp = DveOpSpec(name=op.name, opcode=get_dve_sub_opcode(op.name),
                            uops=lower(spec, ver=ver), rd1_en=has_src1(spec))
            op.uops_sha[ver] = tmp.sha(ver)
        _EQNEG = op
    return _EQNEG


def _build(nc: bass.Bass):
    x_d = nc.dram_tensor("logits", [T_CORE, E], F32, kind="ExternalInput")
    b_d = nc.dram_tensor("bias", [1, E], F32, kind="ExternalInput")
    idx_d = nc.dram_tensor("idx_out", [T_CORE, 8], U32, kind="ExternalOutput")
    w_d = nc.dram_tensor("w_out", [T_CORE, 8], F32, kind="ExternalOutput")

    AX = mybir.AxisListType.X
    OP = mybir.AluOpType
    ACTF = mybir.ActivationFunctionType
    eqneg = _get_eqneg()

    with TileContext(nc) as tc:
        with tc.tile_pool(name="const", bufs=1) as cpool, \
             tc.tile_pool(name="io", bufs=2) as iopool, \
             tc.tile_pool(name="wide", bufs=4) as wpool, \
             tc.tile_pool(name="keep", bufs=2) as kpool, \
             tc.tile_pool(name="slot", bufs=2) as slpool, \
             tc.tile_pool(name="small", bufs=4) as spool:

            biasb = cpool.tile([P, E], F32)
            nc.gpsimd.dma_start(out=biasb[:], in_=b_d[:, :].to_broadcast((P, E)))

            for sp_i in range(N_SUPER):
                m1s = slpool.tile([P, B * G], F32, tag="m1s")
                m2s = slpool.tile([P, B * G], F32, tag="m2s")
                gss = slpool.tile([P, B * G], F32, tag="gss")
                gts = slpool.tile([P, B * G * G], F32, tag="gts")
                ranks = slpool.tile([P, B * G], F32, tag="ranks")
                v8s = slpool.tile([P, B * 8], F32, tag="v8s")
                i8s = slpool.tile([P, B * 8], U32, tag="i8s")
                sv8s = slpool.tile([P, B * 8], F32, tag="sv8s")
                si8s = slpool.tile([P, B * 8], U32, tag="si8s")
                dens = slpool.tile([P, B], F32, tag="dens")
                rdens = slpool.tile([P, B], F32, tag="rdens")
                eqms = slpool.tile([P, B * 64], F32, tag="eqms")
                wms = slpool.tile([P, B * 64], F32, tag="wms")
                w8s = slpool.tile([P, B * 8], F32, tag="w8s")
                wouts = slpool.tile([P, B * 8], F32, tag="wouts")

                # One 1MB load per supertile. Token mapping inside the
                # supertile: partition p, tile b <-> token sp_i*1024 + 8p + b
                # (the output DMAs write the same mapping, so the DRAM
                # result is in natural token order).
                srow = sp_i * B * P
                Ls = iopool.tile([P, B * E], F32, tag="L")
                nc.sync.dma_start(
                    out=Ls[:],
                    in_=x_d[srow:srow + B * P, :].rearrange(
                        "(p x) e -> p (x e)", p=P))

                s_keep = []
                s4c_keep = []
                # ---- phase A: per tile, sigmoid / bias / group top-2 ----
                for b in range(B):
                    L = Ls[:, b * E:(b + 1) * E]

                    e = wpool.tile([P, E], F32, tag="e")
                    nc.scalar.activation(e[:], L, ACTF.Exp, scale=-1.0)
                    u = wpool.tile([P, E], F32, tag="u")
                    nc.scalar.activation(u[:], e[:], ACTF.Copy, bias=1.0)
                    s = kpool.tile([P, E], F32, tag=f"s{b}")
                    nc.vector.reciprocal(s[:], u[:])
                    s_keep.append(s)

                    s4c = kpool.tile([P, E], F32, tag=f"s4c{b}")
                    nc.gpsimd.tensor_tensor(s4c[:], s[:], biasb[:], op=OP.add)
                    s4c_keep.append(s4c)
                    s4c_g = s4c[:].rearrange("p (g e) -> p g e", g=G)

                    m1v = m1s[:, b * G:(b + 1) * G]
                    nc.vector.tensor_reduce(m1v, s4c_g, axis=AX, op=OP.max)
                    t2 = wpool.tile([P, E], F32, tag="t2")
                    nc.vector._custom_dve(
                        eqneg,
                        out=t2[:].rearrange("p (g e) -> p g e", g=G),
                        in0=s4c_g,
                        in1=m1v.unsqueeze(2).broadcast_to([P, G, EG]))
                    nc.vector.tensor_reduce(
                        m2s[:, b * G:(b + 1) * G],
                        t2[:].rearrange("p (g e) -> p g e", g=G),
                        axis=AX, op=OP.max)

                # ---- phase B: batched group ranking ----
                nc.vector.tensor_add(gss[:], m1s[:], m2s[:])
                gs3 = gss[:].rearrange("p (b g) -> p b g", b=B)
                nc.vector.tensor_tensor(
                    gts[:].rearrange("p (b i j) -> p b i j", b=B, i=G),
                    gs3.unsqueeze(2).broadcast_to([P, B, G, G]),
                    gs3.unsqueeze(3).broadcast_to([P, B, G, G]),
                    op=OP.is_gt)
                nc.vector.tensor_reduce(
                    ranks[:], gts[:].rearrange("p (b i j) -> p b i j", b=B, i=G),
                    axis=AX, op=OP.add)

                # ---- phase C: per tile top-8 extraction ----
                for b in range(B):
                    s = s_keep[b]
                    s4c = s4c_keep[b]
                    s4c_g = s4c[:].rearrange("p (g e) -> p g e", g=G)

                    masked = wpool.tile([P, E], F32, tag="masked")
                    rankv = ranks[:, b * G:(b + 1) * G]
                    nc.vector.scalar_tensor_tensor(
                        masked[:].rearrange("p (g e) -> p g e", g=G),
                        rankv.unsqueeze(2).broadcast_to([P, G, EG]), 4.0,
                        s4c_g, op0=OP.is_lt, op1=OP.mult)

                    v8 = v8s[:, b * 8:(b + 1) * 8]
                    nc.vector.max(out=v8, in_=masked[:])
                    nc.vector.max_index(i8s[:, b * 8:(b + 1) * 8], v8, masked[:])

                    marked = wpool.tile([P, E], F32, tag="marked")
                    nc.vector.match_replace(out=marked[:], in_to_replace=v8,
                                            in_values=masked[:], imm_value=BIG)
                    s_sel = wpool.tile([P, E], F32, tag="s_sel")
                    nc.vector.scalar_tensor_tensor(
                        s_sel[:], marked[:], BIG, s[:],
                        op0=OP.is_equal, op1=OP.mult,
                        accum_out=dens[:, b:b + 1])

                    sv8 = sv8s[:, b * 8:(b + 1) * 8]
                    nc.vector.max(out=sv8, in_=s_sel[:])
                    nc.vector.max_index(si8s[:, b * 8:(b + 1) * 8], sv8,
                                        s_sel[:])

                # ---- phase D: batched realign + normalize + store ----
                i8s3 = i8s[:].rearrange("p (b i) -> p b i", b=B)
                si8s3 = si8s[:].rearrange("p (b k) -> p b k", b=B)
                nc.vector.tensor_tensor(
                    eqms[:].rearrange("p (b i k) -> p b i k", b=B, i=8),
                    i8s3.unsqueeze(3).broadcast_to([P, B, 8, 8]),
                    si8s3.unsqueeze(2).broadcast_to([P, B, 8, 8]),
                    op=OP.is_equal)
                sv3 = sv8s[:].rearrange("p (b k) -> p b k", b=B)
                nc.vector.tensor_tensor(
                    wms[:].rearrange("p (b i k) -> p b i k", b=B, i=8),
                    eqms[:].rearrange("p (b i k) -> p b i k", b=B, i=8),
                    sv3.unsqueeze(2).broadcast_to([P, B, 8, 8]),
                    op=OP.mult)
                nc.vector.tensor_reduce(
                    w8s[:], wms[:].rearrange("p (b i k) -> p b i k", b=B, i=8),
                    axis=AX, op=OP.add)
                nc.vector.reciprocal(rdens[:], dens[:])
                rd3 = rdens[:].rearrange("p (b o) -> p b o", b=B)
                nc.vector.scalar_tensor_tensor(
                    wouts[:].rearrange("p (b i) -> p b i", b=B),
                    w8s[:].rearrange("p (b i) -> p b i", b=B), 2.5,
                    rd3.broadcast_to([P, B, 8]),
                    op0=OP.mult, op1=OP.mult)

                nc.sync.dma_start(
                    out=idx_d[srow:srow + B * P, :].rearrange(
                        "(p x) e -> p (x e)", p=P),
                    in_=i8s[:])
                nc.sync.dma_start(
                    out=w_d[srow:srow + B * P, :].rearrange(
                        "(p x) e -> p (x e)", p=P),
                    in_=wouts[:])
    return nc


_COMPILED_NC = None


def _get_nc():
    global _COMPILED_NC
    if _COMPILED_NC is None:
        nc = bacc.Bacc(None, target_bir_lowering=False, debug=False)
        _build(nc)
        nc.finalize()
        _COMPILED_NC = nc
    return _COMPILED_NC


def kernel(router_logits: np.ndarray, correction_bias: np.ndarray,
           trace: bool = False):
    global LAST_EXEC_NS, LAST_RESULTS
    x = np.ascontiguousarray(np.asarray(router_logits), dtype=np.float32)
    b = np.ascontiguousarray(np.asarray(correction_bias),
                             dtype=np.float32).reshape(1, E)
    assert x.shape == (T_FULL, E), x.shape

    nc = _get_nc()
    in_maps = [{"logits": x[c * T_CORE:(c + 1) * T_CORE], "bias": b}
               for c in range(N_CORES)]
    res = run_bass_kernel_spmd(nc, in_maps, core_ids=list(range(N_CORES)),
                               trace=trace)
    LAST_EXEC_NS = res.exec_time_ns
    LAST_RESULTS = res

    idx = np.concatenate([r["idx_out"] for r in res.results], axis=0)
    w = np.concatenate([r["w_out"] for r in res.results], axis=0)
    return idx.view(np.int32), w.astype(np.float32, copy=False)



# revision 5
# speedup vs baseline: 1.0143x; 1.0143x over previous
"""DeepseekV3 top-k router kernel for Trainium2 (Bass/Tile), 8-core SPMD.

Reference computation (per token, 256 experts):
  s    = sigmoid(logits)
  s4c  = s + correction_bias
  group scores = sum of top-2 of s4c within each of 8 groups of 32
  top-4 groups -> mask -> masked s4c
  top-8 of masked s4c -> (indices, values)
  weights = s at those indices, normalized to sum 2.5

Sharding: data-parallel on the token dim across 8 cores (16384 tokens each).
Layout: one token per SBUF partition, its 256 expert scores along the free
dim; 128 tokens per tile, 8 tiles per supertile so the wide elementwise /
reduce stages run as single [128, 2048] instructions (amortizing the ~150
cycle DVE instruction overhead 8x).

Engine split (v4):
  ACT    : sigmoid (fp32 for selection + fp16 copy for the weight gather),
           final reciprocal of the denominators.
  GPSIMD : bias add (s4c), and the weight-extraction scatter chain
           (local_scatter k+1 at i8 -> sub 1 -> local_scatter s16 by slot).
  DVE    : group top-2 (segmented reduce / eqneg custom / reduce), group
           ranking, masking, Max8/FindIndex8 per tile, small batched ops.
The per-tile second extraction of the baseline (match_replace + s_sel +
Max8 + FindIndex8 + 8x8 realign) is replaced by the gpsimd scatter chain,
which pulls ~1.8us/tile off the Vector engine (the bottleneck).
"""
import numpy as np

import concourse.bass as bass
import concourse.bacc as bacc
import concourse.mybir as mybir
from concourse.tile import TileContext
from concourse.bass_utils import run_bass_kernel_spmd

F32 = mybir.dt.float32
F16 = mybir.dt.float16
U32 = mybir.dt.uint32
I16 = mybir.dt.int16

T_FULL = 131072
E = 256
N_CORES = 8
T_CORE = T_FULL // N_CORES      # 16384
P = 128                         # tokens per tile (one per partition)
N_TILES = T_CORE // P           # 128
B = 8                           # tiles per supertile
N_SUPER = N_TILES // B          # 16
G = 8                           # expert groups
EG = E // G                     # experts per group
BIG = 1.0e30

# weight path: 'scatter' = gpsimd local_scatter chain; 'classic' = DVE
# threshold-select + second Max8/FindIndex8 + batched 8x8 realign.
W_PATH = "scatter"
# sigmoid: 'act' = single ACT Sigmoid op; 'exp_recip' = ACT exp + ACT +1 +
# DVE reciprocal_approx_accurate (2 custom DVE passes), in case the ACT
# sigmoid table is too coarse for exact index selection.
SIG_MODE = "act"

LAST_EXEC_NS = None
LAST_RESULTS = None

_EQNEG = None


def _get_eqneg():
    """Fused custom DVE op: out = (in0 == in1) ? -FLT_MAX : in0 (one pass,
    replaces a tensor_tensor(is_equal) + scalar_tensor_tensor pair)."""
    global _EQNEG
    if _EQNEG is None:
        from concourse.dve_ops import (DveOp, OPS, get_dve_sub_opcode,
                                       has_src1)
        from concourse.dve_spec import Spec, Src0, Src1, MaxNeg, select, eq, lower
        from concourse.dve_uop import DveOpSpec
        import concourse.dve_ops as dve_ops_mod

        spec = Spec(
            body=select(eq(Src0, Src1), MaxNeg, Src0),
            reference=lambda in0, in1, s0, s1, imm2: np.where(
                in0 == np.asarray(in1).reshape(np.asarray(in0).shape),
                np.float32(-3.4028234663852886e38), in0
            ).astype(np.float32),
        )
        op = DveOp("RTR_EQNEG", spec, subdim=False, uops_sha={})
        OPS.append(op)
        dve_ops_mod.CUSTOM_DVE_SPECS[op.name] = op.spec
        dve_ops_mod._SUB_OPCODE_FOR_NAME[op.name] = (
            dve_ops_mod._CUSTOM_DVE_ROW_BASE + len(OPS) - 1)
        assert dve_ops_mod._SUB_OPCODE_FOR_NAME[op.name] < 0x20
        for ver in ("v3", "v4"):
            tmp = DveOpSpec(name=op.name, opcode=get_dve_sub_opcode(op.name),
                            uops=lower(spec, ver=ver), rd1_en=has_src1(spec))
            op.uops_sha[ver] = tmp.sha(ver)
        _EQNEG = op
    return _EQNEG


def _build(nc: bass.Bass):
    x_d = nc.dram_tensor("logits", [T_CORE, E], F32, kind="ExternalInput")
    b_d = nc.dram_tensor("bias", [1, E], F32, kind="ExternalInput")
    # offs[j] = 256*((j//8) % 4): column offset of tile j//8 within its
    # 4-tile scatter group.  slotdat[t*8+k] = 16*t + k + 1: slot id (+1 so
    # the post-scatter "-1" turns the zero background into ignored -1s).
    offs_d = nc.dram_tensor("offs", [1, B * 8], U32, kind="ExternalInput")
    slot_d = nc.dram_tensor("slotdat", [1, 32], I16, kind="ExternalInput")
    idx_d = nc.dram_tensor("idx_out", [T_CORE, 8], U32, kind="ExternalOutput")
    w_d = nc.dram_tensor("w_out", [T_CORE, 8], F32, kind="ExternalOutput")

    AX = mybir.AxisListType.X
    OP = mybir.AluOpType
    ACTF = mybir.ActivationFunctionType
    eqneg = _get_eqneg()
    W = B * E                     # 2048 cols per supertile

    with TileContext(nc) as tc:
        with tc.tile_pool(name="const", bufs=1) as cpool, \
             tc.tile_pool(name="io", bufs=2) as iopool, \
             tc.tile_pool(name="wide", bufs=2) as wpool, \
             tc.tile_pool(name="w16", bufs=3) as w16pool, \
             tc.tile_pool(name="slot", bufs=3) as slpool, \
             tc.tile_pool(name="out", bufs=3) as outpool:

            biasb = cpool.tile([P, W], F32)
            for b in range(B):
                nc.gpsimd.dma_start(out=biasb[:, b * E:(b + 1) * E],
                                    in_=b_d[:, :].to_broadcast((P, E)))
            offsb = cpool.tile([P, B * 8], U32)
            nc.gpsimd.dma_start(out=offsb[:],
                                in_=offs_d[:, :].to_broadcast((P, B * 8)))
            slotb = cpool.tile([P, 32], I16)
            nc.gpsimd.dma_start(out=slotb[:],
                                in_=slot_d[:, :].to_broadcast((P, 32)))

            # per-supertile state carried across the software pipeline
            st = {}

            def stage_front(sp):
                """load -> sigmoid -> s4c -> group top-2 -> ranks -> mask ->
                per-tile Max8/FindIndex8 -> scatter-index prep."""
                srow = sp * B * P
                Ls = iopool.tile([P, W], F32, tag="L")
                nc.sync.dma_start(
                    out=Ls[:],
                    in_=x_d[srow:srow + B * P, :].rearrange(
                        "(p x) e -> p (x e)", p=P))

                s32 = wpool.tile([P, W], F32, tag="s32")
                if SIG_MODE == "act":
                    nc.scalar.activation(s32[:], Ls[:], ACTF.Sigmoid)
                else:
                    e_ = wpool.tile([P, W], F32, tag="e")
                    nc.scalar.activation(e_[:], Ls[:], ACTF.Exp, scale=-1.0)
                    u_ = wpool.tile([P, W], F32, tag="u")
                    nc.scalar.activation(u_[:], e_[:], ACTF.Copy, bias=1.0)
                    scr = wpool.tile([P, W], F32, tag="scr")
                    nc.vector.reciprocal_approx_accurate(
                        out=s32[:], in_=u_[:], scratch=scr[:])
                s16 = w16pool.tile([P, W], F16, tag="s16")
                nc.scalar.activation(s16[:], Ls[:], ACTF.Sigmoid)

                s4c = wpool.tile([P, W], F32, tag="s4c")
                nc.gpsimd.tensor_tensor(s4c[:], s32[:], biasb[:], op=OP.add)
                s4c_g = s4c[:].rearrange("p (b g e) -> p b g e", b=B, g=G)

                m1 = slpool.tile([P, B * G], F32, tag="m1")
                nc.vector.tensor_reduce(
                    m1[:].rearrange("p (b g) -> p b g", b=B), s4c_g,
                    axis=AX, op=OP.max)
                t2 = wpool.tile([P, W], F32, tag="t2")
                nc.vector._custom_dve(
                    eqneg,
                    out=t2[:].rearrange("p (q e) -> p q e", q=B * G),
                    in0=s4c[:].rearrange("p (q e) -> p q e", q=B * G),
                    in1=m1[:].rearrange("p q -> p q", q=B * G)
                        .unsqueeze(2).broadcast_to([P, B * G, EG]))
                m2 = slpool.tile([P, B * G], F32, tag="m2")
                nc.vector.tensor_reduce(
                    m2[:].rearrange("p (b g) -> p b g", b=B),
                    t2[:].rearrange("p (b g e) -> p b g e", b=B, g=G),
                    axis=AX, op=OP.max)

                gss = slpool.tile([P, B * G], F32, tag="gss")
                nc.vector.tensor_add(gss[:], m1[:], m2[:])
                gs3 = gss[:].rearrange("p (b g) -> p b g", b=B)
                gts = slpool.tile([P, B * G * G], F32, tag="gts")
                nc.vector.tensor_tensor(
                    gts[:].rearrange("p (b i j) -> p b i j", b=B, i=G),
                    gs3.unsqueeze(2).broadcast_to([P, B, G, G]),
                    gs3.unsqueeze(3).broadcast_to([P, B, G, G]),
                    op=OP.is_gt)
                ranks = slpool.tile([P, B * G], F32, tag="ranks")
                nc.vector.tensor_reduce(
                    ranks[:],
                    gts[:].rearrange("p (b i j) -> p b i j", b=B, i=G),
                    axis=AX, op=OP.add)

                masked = wpool.tile([P, W], F32, tag="masked")
                nc.vector.scalar_tensor_tensor(
                    masked[:].rearrange("p (b g e) -> p b g e", b=B, g=G),
                    ranks[:].rearrange("p (b g) -> p b g", b=B)
                        .unsqueeze(3).broadcast_to([P, B, G, EG]),
                    4.0, s4c_g, op0=OP.is_lt, op1=OP.mult)

                i8s = outpool.tile([P, B * 8], U32, tag="i8s")
                v8s = slpool.tile([P, B * 8], F32, tag="v8s")
                for b in range(B):
                    mb_ = masked[:, b * E:(b + 1) * E]
                    v8 = v8s[:, b * 8:(b + 1) * 8]
                    nc.vector.max(out=v8, in_=mb_)
                    nc.vector.max_index(i8s[:, b * 8:(b + 1) * 8], v8, mb_)

                if W_PATH == "scatter":
                    adj = slpool.tile([P, B * 8], U32, tag="adj")
                    nc.vector.tensor_tensor(adj[:], i8s[:], offsb[:],
                                            op=OP.add)
                    adj16 = slpool.tile([P, B * 8], I16, tag="adj16")
                    nc.vector.tensor_copy(adj16[:], adj[:])
                    st[sp] = dict(s16=s16, adj16=adj16, i8s=i8s, v8s=v8s)
                else:
                    # classic: threshold-select s (>= v8[7]) with fused
                    # denominator accum, then second Max8/FindIndex8.
                    s_sel = wpool.tile([P, W], F32, tag="s_sel")
                    dens = slpool.tile([P, B], F32, tag="dens")
                    sv8s = slpool.tile([P, B * 8], F32, tag="sv8s")
                    si8s = slpool.tile([P, B * 8], U32, tag="si8s")
                    s16 = None
                    for b in range(B):
                        mb_ = masked[:, b * E:(b + 1) * E]
                        nc.vector.scalar_tensor_tensor(
                            s_sel[:, b * E:(b + 1) * E], mb_,
                            v8s[:, b * 8 + 7:b * 8 + 8], s32[:, b * E:(b + 1) * E],
                            op0=OP.is_ge, op1=OP.mult,
                            accum_out=dens[:, b:b + 1])
                        sv8 = sv8s[:, b * 8:(b + 1) * 8]
                        nc.vector.max(out=sv8, in_=s_sel[:, b * E:(b + 1) * E])
                        nc.vector.max_index(si8s[:, b * 8:(b + 1) * 8], sv8,
                                            s_sel[:, b * E:(b + 1) * E])
                    st[sp] = dict(i8s=i8s, sv8s=sv8s, si8s=si8s, dens=dens)

            def stage_scatter(sp):
                """gpsimd: inv[col]=slot+1 at i8 columns; -1; scatter s16."""
                d = st[sp]
                inv = w16pool.tile([P, W], I16, tag="inv")
                for h in range(2):          # two 4-tile halves
                    nc.gpsimd.local_scatter(
                        out_ap=inv[:, h * 1024:(h + 1) * 1024],
                        data_ap=slotb[:],
                        idxs_ap=d["adj16"][:, h * 32:(h + 1) * 32],
                        channels=P, num_elems=1024, num_idxs=32)
                nc.gpsimd.tensor_scalar_sub(inv[:], inv[:], 1)
                w9 = slpool.tile([P, B * 16], F16, tag="w9")
                for h in range(2):
                    nc.gpsimd.local_scatter(
                        out_ap=w9[:, h * 64:(h + 1) * 64],
                        data_ap=d["s16"][:, h * 1024:(h + 1) * 1024],
                        idxs_ap=inv[:, h * 1024:(h + 1) * 1024],
                        channels=P, num_elems=64, num_idxs=1024)
                d["w9"] = w9

            def stage_back(sp):
                """normalize + store."""
                d = st.pop(sp)
                srow = sp * B * P
                wouts = outpool.tile([P, B * 8], F32, tag="wouts")
                rdens = slpool.tile([P, B], F32, tag="rdens")
                if W_PATH == "scatter":
                    w8v = d["w9"][:].rearrange(
                        "p (b s) -> p b s", b=B)[:, :, 0:8]
                    dens = slpool.tile([P, B], F32, tag="dens")
                    nc.vector.tensor_reduce(dens[:], w8v, axis=AX, op=OP.add)
                    nc.vector.reciprocal(rdens[:], dens[:])
                    nc.vector.scalar_tensor_tensor(
                        wouts[:].rearrange("p (b i) -> p b i", b=B),
                        w8v, 2.5,
                        rdens[:].rearrange("p (b o) -> p b o", b=B)
                            .broadcast_to([P, B, 8]),
                        op0=OP.mult, op1=OP.mult)
                else:
                    # realign sv8 (s-value order) to i8 (selection order)
                    eqms = slpool.tile([P, B * 64], F32, tag="eqms")
                    wms = slpool.tile([P, B * 64], F32, tag="wms")
                    w8s = slpool.tile([P, B * 8], F32, tag="w8s")
                    i8s3 = d["i8s"][:].rearrange("p (b i) -> p b i", b=B)
                    si8s3 = d["si8s"][:].rearrange("p (b k) -> p b k", b=B)
                    nc.vector.tensor_tensor(
                        eqms[:].rearrange("p (b i k) -> p b i k", b=B, i=8),
                        i8s3.unsqueeze(3).broadcast_to([P, B, 8, 8]),
                        si8s3.unsqueeze(2).broadcast_to([P, B, 8, 8]),
                        op=OP.is_equal)
                    sv3 = d["sv8s"][:].rearrange("p (b k) -> p b k", b=B)
                    nc.vector.tensor_tensor(
                        wms[:].rearrange("p (b i k) -> p b i k", b=B, i=8),
                        eqms[:].rearrange("p (b i k) -> p b i k", b=B, i=8),
                        sv3.unsqueeze(2).broadcast_to([P, B, 8, 8]),
                        op=OP.mult)
                    nc.vector.tensor_reduce(
                        w8s[:],
                        wms[:].rearrange("p (b i k) -> p b i k", b=B, i=8),
                        axis=AX, op=OP.add)
                    nc.vector.reciprocal(rdens[:], d["dens"][:])
                    nc.vector.scalar_tensor_tensor(
                        wouts[:].rearrange("p (b i) -> p b i", b=B),
                        w8s[:].rearrange("p (b i) -> p b i", b=B), 2.5,
                        rdens[:].rearrange("p (b o) -> p b o", b=B)
                            .broadcast_to([P, B, 8]),
                        op0=OP.mult, op1=OP.mult)

                nc.sync.dma_start(
                    out=idx_d[srow:srow + B * P, :].rearrange(
                        "(p x) e -> p (x e)", p=P),
                    in_=d["i8s"][:])
                nc.sync.dma_start(
                    out=w_d[srow:srow + B * P, :].rearrange(
                        "(p x) e -> p (x e)", p=P),
                    in_=wouts[:])

            lag = 2 if W_PATH == "scatter" else 1
            for sp in range(N_SUPER + lag):
                if sp < N_SUPER:
                    stage_front(sp)
                if W_PATH == "scatter" and 0 <= sp - 1 < N_SUPER:
                    stage_scatter(sp - 1)
                if 0 <= sp - lag < N_SUPER:
                    stage_back(sp - lag)
    return nc


_COMPILED_NC = None


def _get_nc():
    global _COMPILED_NC
    if _COMPILED_NC is None:
        nc = bacc.Bacc(None, target_bir_lowering=False, debug=False)
        _build(nc)
        nc.finalize()
        _COMPILED_NC = nc
    return _COMPILED_NC


def _aux_inputs():
    offs = np.array([[256 * ((j // 8) % 4) for j in range(B * 8)]],
                    dtype=np.uint32)
    slotdat = np.array([[16 * t + k + 1 for t in range(4) for k in range(8)]],
                       dtype=np.int16)
    return offs, slotdat


def kernel(router_logits: np.ndarray, correction_bias: np.ndarray,
           trace: bool = False):
    global LAST_EXEC_NS, LAST_RESULTS
    x = np.ascontiguousarray(np.asarray(router_logits), dtype=np.float32)
    b = np.ascontiguousarray(np.asarray(correction_bias),
                             dtype=np.float32).reshape(1, E)
    assert x.shape == (T_FULL, E), x.shape

    nc = _get_nc()
    offs, slotdat = _aux_inputs()
    in_maps = [{"logits": x[c * T_CORE:(c + 1) * T_CORE], "bias": b,
                "offs": offs, "slotdat": slotdat}
               for c in range(N_CORES)]
    res = run_bass_kernel_spmd(nc, in_maps, core_ids=list(range(N_CORES)),
                               trace=trace)
    LAST_EXEC_NS = res.exec_time_ns
    LAST_RESULTS = res

    idx = np.concatenate([r["idx_out"] for r in res.results], axis=0)
    w = np.concatenate([r["w_out"] for r in res.results], axis=0)
    return idx.view(np.int32), w.astype(np.float32, copy=False)


# revision 13
# speedup vs baseline: 2.1299x; 2.0998x over previous
"""DeepseekV3 top-k router kernel for Trainium2 (Bass/Tile), 8-core SPMD.

Reference computation (per token, 256 experts):
  s    = sigmoid(logits)
  s4c  = s + correction_bias
  group scores = sum of top-2 of s4c within each of 8 groups of 32
  top-4 groups -> mask -> masked s4c
  top-8 of masked s4c -> (indices, values)
  weights = s at those indices, normalized to sum 2.5

Sharding: data-parallel on the token dim across 8 cores (16384 tokens each).
Layout: one token per SBUF partition, its 256 expert scores along the free
dim; 128 tokens per tile, 8 tiles per supertile so the wide elementwise /
reduce stages run as single [128, 2048] instructions (amortizing the ~150
cycle DVE instruction overhead 8x).

Engine split (v4):
  ACT    : sigmoid (fp32 for selection + fp16 copy for the weight gather),
           final reciprocal of the denominators.
  GPSIMD : bias add (s4c), and the weight-extraction scatter chain
           (local_scatter k+1 at i8 -> sub 1 -> local_scatter s16 by slot).
  DVE    : group top-2 (segmented reduce / eqneg custom / reduce), group
           ranking, masking, Max8/FindIndex8 per tile, small batched ops.
The per-tile second extraction of the baseline (match_replace + s_sel +
Max8 + FindIndex8 + 8x8 realign) is replaced by the gpsimd scatter chain,
which pulls ~1.8us/tile off the Vector engine (the bottleneck).
"""
import numpy as np

import concourse.bass as bass
import concourse.bacc as bacc
import concourse.mybir as mybir
from concourse.tile import TileContext
from concourse.bass_utils import run_bass_kernel_spmd

F32 = mybir.dt.float32
F16 = mybir.dt.float16
U32 = mybir.dt.uint32
I16 = mybir.dt.int16

T_FULL = 131072
E = 256
N_CORES = 8
T_CORE = T_FULL // N_CORES      # 16384
P = 128                         # tokens per tile (one per partition)
N_TILES = T_CORE // P           # 128
B = 8                           # tiles per supertile
N_SUPER = N_TILES // B          # 16
G = 8                           # expert groups
EG = E // G                     # experts per group
BIG = 1.0e30

# weight path: 'scatter' = gpsimd local_scatter chain; 'classic' = DVE
# threshold-select + second Max8/FindIndex8 + batched 8x8 realign.
W_PATH = "scatter"
# sigmoid: 'act' = single ACT Sigmoid op; 'exp_recip' = ACT exp + ACT +1 +
# DVE reciprocal_approx_accurate (2 custom DVE passes), in case the ACT
# sigmoid table is too coarse for exact index selection.
SIG_MODE = "act"

LAST_EXEC_NS = None
LAST_RESULTS = None

_EQNEG = None


def _get_eqneg():
    """Fused custom DVE op: out = (in0 == in1) ? -FLT_MAX : in0 (one pass,
    replaces a tensor_tensor(is_equal) + scalar_tensor_tensor pair)."""
    global _EQNEG
    if _EQNEG is None:
        from concourse.dve_ops import (DveOp, OPS, get_dve_sub_opcode,
                                       has_src1)
        from concourse.dve_spec import Spec, Src0, Src1, MaxNeg, select, eq, lower
        from concourse.dve_uop import DveOpSpec
        import concourse.dve_ops as dve_ops_mod

        spec = Spec(
            body=select(eq(Src0, Src1), MaxNeg, Src0),
            reference=lambda in0, in1, s0, s1, imm2: np.where(
                in0 == np.asarray(in1).reshape(np.asarray(in0).shape),
                np.float32(-3.4028234663852886e38), in0
            ).astype(np.float32),
        )
        op = DveOp("RTR_EQNEG", spec, subdim=False, uops_sha={})
        OPS.append(op)
        dve_ops_mod.CUSTOM_DVE_SPECS[op.name] = op.spec
        dve_ops_mod._SUB_OPCODE_FOR_NAME[op.name] = (
            dve_ops_mod._CUSTOM_DVE_ROW_BASE + len(OPS) - 1)
        assert dve_ops_mod._SUB_OPCODE_FOR_NAME[op.name] < 0x20
        for ver in ("v3", "v4"):
            tmp = DveOpSpec(name=op.name, opcode=get_dve_sub_opcode(op.name),
                            uops=lower(spec, ver=ver), rd1_en=has_src1(spec))
            op.uops_sha[ver] = tmp.sha(ver)
        _EQNEG = op
    return _EQNEG


def _build(nc: bass.Bass):
    x_d = nc.dram_tensor("logits", [T_CORE, E], F32, kind="ExternalInput")
    b_d = nc.dram_tensor("bias", [1, E], F32, kind="ExternalInput")
    # offs[j] = 256*((j//8) % 4): column offset of tile j//8 within its
    # 4-tile scatter group.  slotdat[t*8+k] = 16*t + k + 1: slot id (+1 so
    # the post-scatter "-1" turns the zero background into ignored -1s).
    offs_d = nc.dram_tensor("offs", [1, B * 8], U32, kind="ExternalInput")
    slot_d = nc.dram_tensor("slotdat", [1, 32], I16, kind="ExternalInput")
    idx_d = nc.dram_tensor("idx_out", [T_CORE, 8], U32, kind="ExternalOutput")
    w_d = nc.dram_tensor("w_out", [T_CORE, 8], F32, kind="ExternalOutput")

    AX = mybir.AxisListType.X
    OP = mybir.AluOpType
    ACTF = mybir.ActivationFunctionType
    eqneg = _get_eqneg()
    W = B * E                     # 2048 cols per supertile

    with TileContext(nc) as tc:
        with tc.tile_pool(name="const", bufs=1) as cpool, \
             tc.tile_pool(name="io", bufs=2) as iopool, \
             tc.tile_pool(name="wide", bufs=2) as wpool, \
             tc.tile_pool(name="w16", bufs=3) as w16pool, \
             tc.tile_pool(name="slot", bufs=3) as slpool, \
             tc.tile_pool(name="out", bufs=3) as outpool:

            biasb = cpool.tile([P, W], F32)
            for b in range(B):
                nc.gpsimd.dma_start(out=biasb[:, b * E:(b + 1) * E],
                                    in_=b_d[:, :].to_broadcast((P, E)))
            offsb = cpool.tile([P, B * 8], U32)
            nc.gpsimd.dma_start(out=offsb[:],
                                in_=offs_d[:, :].to_broadcast((P, B * 8)))
            slotb = cpool.tile([P, 32], I16)
            nc.gpsimd.dma_start(out=slotb[:],
                                in_=slot_d[:, :].to_broadcast((P, 32)))

            # per-supertile state carried across the software pipeline
            st = {}

            def stage_load(sp):
                """DMA load + ACT sigmoids for supertile sp (runs one
                iteration ahead of the DVE stage)."""
                srow = sp * B * P
                Ls = iopool.tile([P, W], F32, tag="L")
                nc.sync.dma_start(
                    out=Ls[:],
                    in_=x_d[srow:srow + B * P, :].rearrange(
                        "(p x) e -> p (x e)", p=P))

                s32 = wpool.tile([P, W], F32, tag="s32")
                if SIG_MODE == "act":
                    nc.scalar.activation(s32[:], Ls[:], ACTF.Sigmoid)
                else:
                    e_ = wpool.tile([P, W], F32, tag="e")
                    nc.scalar.activation(e_[:], Ls[:], ACTF.Exp, scale=-1.0)
                    u_ = wpool.tile([P, W], F32, tag="u")
                    nc.scalar.activation(u_[:], e_[:], ACTF.Copy, bias=1.0)
                    scr = wpool.tile([P, W], F32, tag="scr")
                    nc.vector.reciprocal_approx_accurate(
                        out=s32[:], in_=u_[:], scratch=scr[:])
                s16 = w16pool.tile([P, W], F16, tag="s16")
                nc.scalar.activation(s16[:], Ls[:], ACTF.Sigmoid)
                st[sp] = dict(s32=s32, s16=s16)

            def stage_s4c(sp):
                """gpsimd bias add (one iteration ahead of the DVE stage, so
                the trio never waits on it)."""
                s4c = wpool.tile([P, W], F32, tag="s4c")
                nc.gpsimd.tensor_tensor(s4c[:], st[sp]["s32"][:], biasb[:],
                                        op=OP.add)
                st[sp]["s4c"] = s4c

            def stage_front(sp, mid_cb=None):
                """DVE: group top-2 -> ranks -> mask -> per-tile
                Max8/FindIndex8 -> scatter-index prep.  mid_cb (if set)
                emits the previous supertile's scatter-chain sub/sc2 at the
                point where sc1's output is ready, keeping both queues hot."""
                s32 = st[sp]["s32"]
                s16 = st[sp]["s16"]
                s4c = st[sp]["s4c"]
                s4c_g = s4c[:].rearrange("p (b g e) -> p b g e", b=B, g=G)

                m1 = slpool.tile([P, B * G], F32, tag="m1")
                nc.vector.tensor_reduce(
                    m1[:].rearrange("p (b g) -> p b g", b=B), s4c_g,
                    axis=AX, op=OP.max)
                t2 = wpool.tile([P, W], F32, tag="t2")
                nc.vector._custom_dve(
                    eqneg,
                    out=t2[:].rearrange("p (q e) -> p q e", q=B * G),
                    in0=s4c[:].rearrange("p (q e) -> p q e", q=B * G),
                    in1=m1[:].rearrange("p q -> p q", q=B * G)
                        .unsqueeze(2).broadcast_to([P, B * G, EG]))
                m2 = slpool.tile([P, B * G], F32, tag="m2")
                nc.vector.tensor_reduce(
                    m2[:].rearrange("p (b g) -> p b g", b=B),
                    t2[:].rearrange("p (b g e) -> p b g e", b=B, g=G),
                    axis=AX, op=OP.max)

                gss = slpool.tile([P, B * G], F32, tag="gss")
                nc.vector.tensor_add(gss[:], m1[:], m2[:])
                gs3 = gss[:].rearrange("p (b g) -> p b g", b=B)
                gts = slpool.tile([P, B * G * G], F32, tag="gts")
                nc.vector.tensor_tensor(
                    gts[:].rearrange("p (b i j) -> p b i j", b=B, i=G),
                    gs3.unsqueeze(2).broadcast_to([P, B, G, G]),
                    gs3.unsqueeze(3).broadcast_to([P, B, G, G]),
                    op=OP.is_gt)
                ranks = slpool.tile([P, B * G], F32, tag="ranks")
                nc.vector.tensor_reduce(
                    ranks[:],
                    gts[:].rearrange("p (b i j) -> p b i j", b=B, i=G),
                    axis=AX, op=OP.add)

                if mid_cb is not None:
                    mid_cb()
                masked = wpool.tile([P, W], F32, tag="masked")
                nc.vector.scalar_tensor_tensor(
                    masked[:].rearrange("p (q e) -> p q e", q=B * G),
                    ranks[:].unsqueeze(2).broadcast_to([P, B * G, EG]),
                    4.0, s4c[:].rearrange("p (q e) -> p q e", q=B * G),
                    op0=OP.is_lt, op1=OP.mult)

                i8s = outpool.tile([P, B * 8], U32, tag="i8s")
                v8s = slpool.tile([P, B * 8], F32, tag="v8s")
                for b in range(B):
                    mb_ = masked[:, b * E:(b + 1) * E]
                    v8 = v8s[:, b * 8:(b + 1) * 8]
                    nc.vector.max(out=v8, in_=mb_)
                    nc.vector.max_index(i8s[:, b * 8:(b + 1) * 8], v8, mb_)

                if W_PATH == "scatter":
                    adj = slpool.tile([P, B * 8], U32, tag="adj")
                    nc.vector.tensor_tensor(adj[:], i8s[:], offsb[:],
                                            op=OP.add)
                    adj16 = slpool.tile([P, B * 8], I16, tag="adj16")
                    nc.vector.tensor_copy(adj16[:], adj[:])
                    st[sp].update(adj16=adj16, i8s=i8s, v8s=v8s)
                else:
                    # classic: threshold-select s (>= v8[7]) with fused
                    # denominator accum, then second Max8/FindIndex8.
                    s_sel = wpool.tile([P, W], F32, tag="s_sel")
                    dens = slpool.tile([P, B], F32, tag="dens")
                    sv8s = slpool.tile([P, B * 8], F32, tag="sv8s")
                    si8s = slpool.tile([P, B * 8], U32, tag="si8s")
                    for b in range(B):
                        mb_ = masked[:, b * E:(b + 1) * E]
                        nc.vector.scalar_tensor_tensor(
                            s_sel[:, b * E:(b + 1) * E], mb_,
                            v8s[:, b * 8 + 7:b * 8 + 8], s32[:, b * E:(b + 1) * E],
                            op0=OP.is_ge, op1=OP.mult,
                            accum_out=dens[:, b:b + 1])
                        sv8 = sv8s[:, b * 8:(b + 1) * 8]
                        nc.vector.max(out=sv8, in_=s_sel[:, b * E:(b + 1) * E])
                        nc.vector.max_index(si8s[:, b * 8:(b + 1) * 8], sv8,
                                            s_sel[:, b * E:(b + 1) * E])
                    st[sp].update(i8s=i8s, sv8s=sv8s, si8s=si8s, dens=dens)

            def scatter_sc1(sp):
                """gpsimd: inv[col] = slot+1 at the i8 columns."""
                d = st[sp]
                inv = w16pool.tile([P, W], I16, tag="inv")
                for h in range(2):          # two 4-tile halves
                    nc.gpsimd.local_scatter(
                        out_ap=inv[:, h * 1024:(h + 1) * 1024],
                        data_ap=slotb[:],
                        idxs_ap=d["adj16"][:, h * 32:(h + 1) * 32],
                        channels=P, num_elems=1024, num_idxs=32)
                d["inv"] = inv

            def scatter_fin(sp):
                """inv -= 1 (DVE: int16 on the Q7 cores hits a ~17cyc/elem
                scalar path, 29us/op), then gpsimd scatters s16 by slot."""
                d = st[sp]
                inv = d["inv"]
                nc.vector.tensor_scalar_sub(inv[:], inv[:], 1)
                w9 = slpool.tile([P, B * 16], F16, tag="w9")
                for h in range(2):
                    nc.gpsimd.local_scatter(
                        out_ap=w9[:, h * 64:(h + 1) * 64],
                        data_ap=d["s16"][:, h * 1024:(h + 1) * 1024],
                        idxs_ap=inv[:, h * 1024:(h + 1) * 1024],
                        channels=P, num_elems=64, num_idxs=1024)
                d["w9"] = w9

            def stage_back(sp):
                """normalize + store."""
                d = st.pop(sp)
                srow = sp * B * P
                wouts = outpool.tile([P, B * 8], F32, tag="wouts")
                rdens = slpool.tile([P, B], F32, tag="rdens")
                if W_PATH == "scatter":
                    w8v = d["w9"][:].rearrange(
                        "p (b s) -> p b s", b=B)[:, :, 0:8]
                    dens = slpool.tile([P, B], F32, tag="dens")
                    nc.vector.tensor_reduce(dens[:], w8v, axis=AX, op=OP.add)
                    nc.vector.reciprocal(rdens[:], dens[:])
                    nc.vector.scalar_tensor_tensor(
                        wouts[:].rearrange("p (b i) -> p b i", b=B),
                        w8v, 2.5,
                        rdens[:].rearrange("p (b o) -> p b o", b=B)
                            .broadcast_to([P, B, 8]),
                        op0=OP.mult, op1=OP.mult)
                else:
                    # realign sv8 (s-value order) to i8 (selection order)
                    eqms = slpool.tile([P, B * 64], F32, tag="eqms")
                    wms = slpool.tile([P, B * 64], F32, tag="wms")
                    w8s = slpool.tile([P, B * 8], F32, tag="w8s")
                    i8s3 = d["i8s"][:].rearrange("p (b i) -> p b i", b=B)
                    si8s3 = d["si8s"][:].rearrange("p (b k) -> p b k", b=B)
                    nc.vector.tensor_tensor(
                        eqms[:].rearrange("p (b i k) -> p b i k", b=B, i=8),
                        i8s3.unsqueeze(3).broadcast_to([P, B, 8, 8]),
                        si8s3.unsqueeze(2).broadcast_to([P, B, 8, 8]),
                        op=OP.is_equal)
                    sv3 = d["sv8s"][:].rearrange("p (b k) -> p b k", b=B)
                    nc.vector.tensor_tensor(
                        wms[:].rearrange("p (b i k) -> p b i k", b=B, i=8),
                        eqms[:].rearrange("p (b i k) -> p b i k", b=B, i=8),
                        sv3.unsqueeze(2).broadcast_to([P, B, 8, 8]),
                        op=OP.mult)
                    nc.vector.tensor_reduce(
                        w8s[:],
                        wms[:].rearrange("p (b i k) -> p b i k", b=B, i=8),
                        axis=AX, op=OP.add)
                    nc.vector.reciprocal(rdens[:], d["dens"][:])
                    nc.vector.scalar_tensor_tensor(
                        wouts[:].rearrange("p (b i) -> p b i", b=B),
                        w8s[:].rearrange("p (b i) -> p b i", b=B), 2.5,
                        rdens[:].rearrange("p (b o) -> p b o", b=B)
                            .broadcast_to([P, B, 8]),
                        op0=OP.mult, op1=OP.mult)

                nc.sync.dma_start(
                    out=idx_d[srow:srow + B * P, :].rearrange(
                        "(p x) e -> p (x e)", p=P),
                    in_=d["i8s"][:])
                nc.sync.dma_start(
                    out=w_d[srow:srow + B * P, :].rearrange(
                        "(p x) e -> p (x e)", p=P),
                    in_=wouts[:])

            stage_load(0)
            stage_s4c(0)
            lag = 2 if W_PATH == "scatter" else 1
            for it in range(N_SUPER + lag):
                if it + 1 < N_SUPER:
                    stage_load(it + 1)
                if W_PATH == "scatter" and 0 <= it - 1 < N_SUPER:
                    scatter_sc1(it - 1)

                    def mid_cb(sp_prev=it - 1):
                        scatter_fin(sp_prev)
                else:
                    mid_cb = None
                if it < N_SUPER:
                    stage_front(it, mid_cb)
                elif mid_cb is not None:
                    mid_cb()          # drain: no front stage left
                if it + 1 < N_SUPER:
                    stage_s4c(it + 1)
                if 0 <= it - lag < N_SUPER:
                    stage_back(it - lag)
    return nc


_COMPILED_NC = None


def _get_nc():
    global _COMPILED_NC
    if _COMPILED_NC is None:
        nc = bacc.Bacc(None, target_bir_lowering=False, debug=False)
        _build(nc)
        nc.finalize()
        _COMPILED_NC = nc
    return _COMPILED_NC


def _aux_inputs():
    offs = np.array([[256 * ((j // 8) % 4) for j in range(B * 8)]],
                    dtype=np.uint32)
    slotdat = np.array([[16 * t + k + 1 for t in range(4) for k in range(8)]],
                       dtype=np.int16)
    return offs, slotdat


def kernel(router_logits: np.ndarray, correction_bias: np.ndarray,
           trace: bool = False):
    global LAST_EXEC_NS, LAST_RESULTS
    x = np.ascontiguousarray(np.asarray(router_logits), dtype=np.float32)
    b = np.ascontiguousarray(np.asarray(correction_bias),
                             dtype=np.float32).reshape(1, E)
    assert x.shape == (T_FULL, E), x.shape

    nc = _get_nc()
    offs, slotdat = _aux_inputs()
    in_maps = [{"logits": x[c * T_CORE:(c + 1) * T_CORE], "bias": b,
                "offs": offs, "slotdat": slotdat}
               for c in range(N_CORES)]
    res = run_bass_kernel_spmd(nc, in_maps, core_ids=list(range(N_CORES)),
                               trace=trace)
    LAST_EXEC_NS = res.exec_time_ns
    LAST_RESULTS = res

    idx = np.concatenate([r["idx_out"] for r in res.results], axis=0)
    w = np.concatenate([r["w_out"] for r in res.results], axis=0)
    return idx.view(np.int32), w.astype(np.float32, copy=False)
